# revision 22
# baseline (speedup 1.0000x reference)
"""Trainium2 Bass kernel for the binarized 2-layer MLP (nn_FC_small).

Network (reference semantics):
    h  = sign(x) @ sign(W1).T            # [B, 512], B = 65536, in = 768
    h  = batchnorm(h, g1, b1)            # training-mode, full-batch stats
    h  = clip(h, -1, 1)                  # hardtanh (sign-preserving)
    o  = sign(h) @ sign(W2).T            # [B, 10]
    o  = batchnorm(o, g2, b2)

Key identities:
  * sign(clip(z)) == sign(z); with g>0: sign(BN(h)) == sign(h - T),
    T = mu - b*sd/g.
  * Codes: {-.5,+.5} via DVE/Pool dual-op tensor_scalar ((x>0) - 0.5,
    weights +-2) or +-1 via ACT sign (weights +-1). Both give exact
    sign(x)*sign(w) products -- no per-column constants at all.

Performance structure (per core, batch 8192; engine budgets vs the
~76us x-stream DMA floor):
  - x streamed RAW f32 on sync+gpsimd queues; stage pool 4 bufs so the
    stream never stalls on a consumer hiccup.
  - fp8 codes in natural layout with byte-strided outs: byte pairs
    (feat f, feat f+384) form uint16 words; PE transposes on the uint16
    view feed fp8 DoubleRow mm1 directly.  DVE codes byte0 (dual-op,
    2x SBUF mode ~13us), ACT codes byte1 (sign, ~21us).
  - tp-evac PSUM->SBUF all on DVE (uint16 2x mode, ~13us).
  - mm1 evac PSUM->h1s f16 all on ACT (~32us).
  - bn_stats on the f16 h1s copy, [128,1024] per h-block (~36us DVE),
    keeping DVE ~62us < DMA floor.
  - Stats (mean,var) AllGather with a tight, high-priority launch chain;
    ncfw warmed up by a dummy AllGather at kernel start.
  - Phase B: s8 codes split DVE/ACT/Pool into fp8 byte pairs; mm2 is
    fp8 DoubleRow (2 matmuls per bc); mm2 evac via DMA; bn_stats per
    2048-slab; final affine split across DVE/ACT/Pool and output DMA
    split over 4 queues.

Sharding: data-parallel over batch across 8 cores (8192 rows each).
Output returned transposed [10, 8192] per core; host concatenates + .T.
"""
import numpy as np

import concourse.bass as bass
import concourse.bacc as bacc
import concourse.tile as tile
import concourse.mybir as mybir
from concourse import bass_utils
from concourse.masks import make_identity

F32 = mybir.dt.float32
F16 = mybir.dt.float16
BF16 = mybir.dt.bfloat16
FP8 = mybir.dt.float8e4
U16 = mybir.dt.uint16
GT = mybir.AluOpType.is_gt
MUL = mybir.AluOpType.mult
ADD = mybir.AluOpType.add
SUB = mybir.AluOpType.subtract
DR = mybir.MatmulPerfMode.DoubleRow
IDENT = mybir.ActivationFunctionType.Identity
SIGN = mybir.ActivationFunctionType.Sign

N_CORES = 8
B = 65536
IND, HID, OUT = 768, 512, 10
B_LOC = B // N_CORES          # 8192
BC = 512                      # batch chunk
NBC = B_LOC // BC             # 16
NT = 3                        # pair-plane tiles (384 pairs = 3*128)
HC = HID // 128               # 4 hid-chunks
EPS = 1e-5

_cache = {}


def build():
    if "nc" in _cache:
        return _cache["nc"]
    nc = bacc.Bacc("TRN2", target_bir_lowering=False, debug=False,
                   num_devices=N_CORES)
    x = nc.dram_tensor("x", [B_LOC, IND], F32, kind="ExternalInput")
    w1 = nc.dram_tensor("w1", [HID, IND], F32, kind="ExternalInput")
    w2 = nc.dram_tensor("w2", [OUT, HID], F32, kind="ExternalInput")
    g1 = nc.dram_tensor("g1", [HID], F32, kind="ExternalInput")
    b1 = nc.dram_tensor("b1", [HID], F32, kind="ExternalInput")
    g2 = nc.dram_tensor("g2", [OUT], F32, kind="ExternalInput")
    b2 = nc.dram_tensor("b2", [OUT], F32, kind="ExternalInput")
    o_out = nc.dram_tensor("o_out", [OUT, B_LOC], F32, kind="ExternalOutput")

    with tile.TileContext(nc) as tc:
        with (
            tc.tile_pool(name="cst", bufs=1) as cst,
            tc.tile_pool(name="stage", bufs=3) as stage,
            tc.tile_pool(name="wpool", bufs=1) as wpool,
            tc.tile_pool(name="code", bufs=2) as codep,
            tc.tile_pool(name="s8p", bufs=2) as s8p,
            tc.tile_pool(name="pst", bufs=2, space="PSUM") as pst,    # transposes
            tc.tile_pool(name="psm", bufs=3, space="PSUM") as psm,    # mm1/mm2
            tc.tile_pool(name="dram", bufs=1, space="DRAM") as dpool,
        ):
            ident = cst.tile([128, 128], BF16)
            make_identity(nc, ident[:])
            bias_m1 = cst.tile([128, 1], F32)
            nc.vector.memset(bias_m1[:], -1.0)
            bias_m2 = cst.tile([128, 1], F32)
            nc.vector.memset(bias_m2[:], -2.0)
            wup_sb = cst.tile([128, 1], F32)

            # x-chunk DMAs alternate sync / gpsimd issue queues; both are
            # otherwise light, and a single queue bubbles the stream on
            # descriptor-generation gaps.
            def x_dma(bc_, xt_):
                eng = nc.gpsimd if bc_ % 2 == 0 else nc.sync
                eng.dma_start(
                    out=xt_[:],
                    in_=x.ap()[bc_ * BC:(bc_ + 1) * BC].rearrange("(s p) f -> p s f", p=128))

            xcs = {}
            for pc in range(3):
                xt = stage.tile([128, 4, IND], F32, tag="xc")
                x_dma(pc, xt)
                xcs[pc] = xt

            # codes: byte0 (feat 0..383)   = (x>0) - 0.5 on DVE (w' = +-2)
            #        byte1 (feat 384..767) = sign(x)     on ACT (w' = +-1)
            def make_codes(cdt_, xc_):
                cdtv = cdt_[:].rearrange("p s (q j) -> p s q j", j=2)
                nc.vector.tensor_scalar(
                    cdtv[:, :, :, 0], xc_[:, :, 0:384], 0.0, 0.5, GT, SUB)
                nc.scalar.sign(cdtv[:, :, :, 1], xc_[:, :, 384:768])

            cds = {}
            for pc in range(2):
                cdt = codep.tile([128, 4, IND], FP8, tag="cd")
                make_codes(cdt, xcs[pc])
                cds[pc] = cdt

            # ---------------- weight prep ----------------
            # W1 [512, 768] -> {0,1} codes -> pair-layout fp8
            # w1sT[ki, t, ko, hid]: ko=0 -> +-2 (scale 4 bias -2);
            #                       ko=1 -> +-1 (scale 2 bias -1)
            w1c = wpool.tile([128, 4, IND], F32)
            nc.sync.dma_start(out=w1c[:], in_=w1.ap().rearrange("(c p) f -> p c f", p=128))
            w1b = wpool.tile([128, 4, IND], BF16)
            nc.vector.tensor_scalar(w1b[:], w1c[:], 0.0, None, GT)
            w1sT = cst.tile([128, NT, 2, HID], FP8)   # 3 KB/part
            for k in range(6):
                t, ko = (k, 0) if k < 3 else (k - 3, 1)
                sc, bi = (4.0, bias_m2) if k < 3 else (2.0, bias_m1)
                pw = psm.tile([128, HC, 128], BF16, tag="mm")
                for c in range(HC):
                    nc.tensor.transpose(pw[:, c, :], w1b[:, c, k * 128:(k + 1) * 128], ident[:])
                nc.scalar.activation(w1sT[:, t, ko, :], pw[:].rearrange("p c f -> p (c f)"),
                                     IDENT, bias=bi[:], scale=sc)

            # ---------------- persistent buffers ----------------
            xT8 = cst.tile([128, NT, B_LOC], U16, tag="bigx")   # 48 KB/part
            h1s = cst.tile([128, HC, B_LOC], F16, tag="bigh")   # 64 KB/part
            st1 = cst.tile([128, HC, NBC * 6], F32)
            h2T = cst.tile([OUT, B_LOC], F32, tag="bigx")       # reuses xT8 slab
            s2sum = cst.tile([OUT, NBC], F32)
            s2sq = cst.tile([OUT, NBC], F32)
            scrA = cst.tile([OUT, BC], F32)
            scrD = cst.tile([OUT, BC], F32)

            # warm-up collective: pays the ncfw cold-start barrier during
            # phase A.  The prefetch DMAs above are already enqueued on the
            # gpsimd queue, so the doorbell stall overlaps chunk 0-2 compute.
            wloc = dpool.tile([128, 1], F32)
            wgat = dpool.tile([128 * N_CORES, 1], F32)
            with tc.high_priority():
                nc.vector.memset(wup_sb[:], 0.0)
                nc.sync.dma_start(out=wloc[:], in_=wup_sb[:])
                nc.gpsimd.collective_compute(
                    "AllGather", mybir.AluOpType.bypass,
                    ins=[wloc.opt()], outs=[wgat.opt()],
                    replica_groups=[list(range(N_CORES))])

            # ---------------- phase A ----------------
            # per chunk: DMA f32 -> codes (byte-strided fp8) -> uint16 pair
            # transposes -> DVE evac into xT8.  mm1 h-blocks of the PREVIOUS
            # group are emitted between transpose halves so real-matmul
            # activity never pauses long enough for the PE HAM clock gate
            # to re-throttle.  Stats run on the f16 h1s copy (not PSUM).
            def mm_h_block(g, h):
                bs0 = 2 * g * BC
                mp = psm.tile([128, 2, BC], F32, tag="mm")
                for t in range(NT):
                    lw = w1sT[:, t, :, h * 128:(h + 1) * 128]
                    for j in range(2):
                        rhs = xT8[:, t, bs0 + j * BC: bs0 + (j + 1) * BC] \
                            .bitcast(FP8).rearrange("p (n j) -> p j n", j=2)
                        nc.tensor.matmul(
                            mp[:, j, :], lw, rhs,
                            start=(t == 0), stop=(t == NT - 1),
                            perf_mode=DR)
                mpw = mp[:].rearrange("p j n -> p (j n)")
                nc.scalar.copy(h1s[:, h, bs0:bs0 + 2 * BC], mpw)
                for j in range(2):
                    bc2 = 2 * g + j
                    nc.vector.bn_stats(st1[:, h, bc2 * 6:(bc2 + 1) * 6],
                                       h1s[:, h, bs0 + j * BC:bs0 + (j + 1) * BC])

            with nc.named_scope("phaseA"):
                for grp in range(NBC // 2):
                    hb = 0
                    for half2 in range(2):
                        bc = 2 * grp + half2
                        bs = bc * BC
                        if bc in xcs:
                            xc = xcs.pop(bc)
                        else:
                            xc = stage.tile([128, 4, IND], F32, tag="xc")
                            x_dma(bc, xc)
                        if bc + 2 < NBC and bc + 2 not in xcs:
                            # keep the DMA queue primed ~3 chunks ahead
                            nxt = stage.tile([128, 4, IND], F32, tag="xc")
                            x_dma(bc + 2, nxt)
                            xcs[bc + 2] = nxt
                        if bc in cds:
                            cd = cds.pop(bc)
                        else:
                            cd = codep.tile([128, 4, IND], FP8, tag="cd")
                            make_codes(cd, xc)
                        for half in range(2):
                            # mm block FIRST: it is ready to run while the
                            # transposes below wait on this chunk's DMA and
                            # codes — avoids head-of-line blocking on the
                            # PE's strict-FIFO queue.
                            if grp > 0:
                                mm_h_block(grp - 1, hb)
                                hb += 1
                            tp = pst.tile([128, NT, 2, 128], BF16, tag="tp")
                            for t in range(NT):
                                for s2 in range(2):
                                    s = 2 * half + s2
                                    cdu = cd[:, s, :].bitcast(BF16)
                                    nc.tensor.transpose(
                                        tp[:, t, s2, :],
                                        cdu[:, t * 128:(t + 1) * 128],
                                        ident[:])
                            # one evac per half-chunk, uint16 2x mode on DVE
                            ev_out = xT8[:, :, bs + half * 256: bs + half * 256 + 256] \
                                .rearrange("p t (s b) -> p t s b", s=2).bitcast(BF16)
                            nc.vector.tensor_copy(ev_out, tp[:])
            # ---------------- local stats -> AllGather ----------------
            agg1 = cst.tile([128, HC, 2], F32)
            loc1 = dpool.tile([128, HC * 2], F32)
            gat1 = dpool.tile([128 * N_CORES, HC * 2], F32)
            ga1 = cst.tile([128, N_CORES, HC * 2], F32)
            with nc.named_scope("phaseA"):
                for h in range(HC):
                    mm_h_block(NBC // 2 - 1, h)
                    with tc.high_priority():
                        nc.vector.bn_aggr(agg1[:, h, :],
                                          st1[:, h, :].rearrange("p (n s) -> p n s", s=6))
            with tc.high_priority():
                nc.gpsimd.dma_start(out=loc1[:], in_=agg1[:].rearrange("p c s -> p (c s)"))
                nc.gpsimd.collective_compute(
                    "AllGather", mybir.AluOpType.bypass,
                    ins=[loc1.opt()], outs=[gat1.opt()],
                    replica_groups=[list(range(N_CORES))])
                nc.sync.dma_start(out=ga1[:], in_=gat1[:].rearrange("(c p) s -> p c s", p=128))

            # W2 prep + g/b vectors (overlaps phase A / collective)
            # chunk -> (pair, ko): c0->(0,0) +-2; c1->(0,1) +-1;
            #                      c2->(1,0) +-1; c3->(1,1) +-2
            w2n = cst.tile([OUT, HID], F32)
            nc.sync.dma_start(out=w2n[:], in_=w2.ap())
            w2b = cst.tile([OUT, HID], BF16)
            nc.vector.tensor_scalar(w2b[:], w2n[:], 0.0, None, GT)
            # all four chunks coded {+-.5} (h0/h1/h2 DVE, h3 Pool) -> w2 +-2.
            # Stationary padded to 16 cols (DoubleRow needs >=16); pad = 0.
            OUTP = 16
            w2sT = cst.tile([128, 2, 2, OUTP], FP8)
            nc.vector.memset(w2sT[:], 0.0)
            W2MAP = [(0, 0, 4.0), (0, 1, 4.0), (1, 0, 4.0), (1, 1, 4.0)]
            for c in range(HC):
                pw2 = psm.tile([128, OUT], BF16, tag="mm")
                nc.tensor.transpose(pw2[:], w2b[:, c * 128:(c + 1) * 128], ident[:OUT, :OUT])
                pr, ko, sc = W2MAP[c]
                bi = bias_m2 if sc == 4.0 else bias_m1
                nc.scalar.activation(w2sT[:, pr, ko, 0:OUT], pw2[:], IDENT, bias=bi[:], scale=sc)
            g1c = cst.tile([128, HC], F32)
            b1c = cst.tile([128, HC], F32)
            for c in range(HC):
                nc.sync.dma_start(out=g1c[:, c:c + 1], in_=g1.ap()[c * 128:(c + 1) * 128])
                nc.sync.dma_start(out=b1c[:, c:c + 1], in_=b1.ap()[c * 128:(c + 1) * 128])
            g2c = cst.tile([OUT, 1], F32)
            b2c = cst.tile([OUT, 1], F32)
            nc.sync.dma_start(out=g2c[:], in_=g2.ap())
            nc.sync.dma_start(out=b2c[:], in_=b2.ap())
            # corr_pre = b1/g1 does not depend on the collective result
            ig1 = cst.tile([128, HC], F32)
            nc.vector.reciprocal(ig1[:], g1c[:])
            corr_pre = cst.tile([128, HC], F32)
            nc.vector.tensor_tensor(corr_pre[:], b1c[:], ig1[:], MUL)

            # combine: mean_tot = avg(mean_c); var_tot = avg(var_c + mean_c^2) - mean_tot^2
            with nc.named_scope("combine1"):
                q1 = cst.tile([128, N_CORES, HC * 2], F32)
                nc.vector.tensor_tensor(q1[:], ga1[:], ga1[:], MUL)
                msum = cst.tile([128, HC * 2], F32)
                qsum = cst.tile([128, HC * 2], F32)
                nc.vector.tensor_reduce(msum[:], ga1[:].rearrange("p c s -> p s c"),
                                        mybir.AxisListType.X, ADD)
                nc.vector.tensor_reduce(qsum[:], q1[:].rearrange("p c s -> p s c"),
                                        mybir.AxisListType.X, ADD)
                m1 = cst.tile([128, HC], F32)
                mview = msum[:].rearrange("p (c s) -> p c s", s=2)
                qview = qsum[:].rearrange("p (c s) -> p c s", s=2)
                nc.vector.tensor_scalar(m1[:], mview[:, :, 0], 1.0 / N_CORES, None, MUL)
                e2 = cst.tile([128, HC], F32)
                nc.vector.tensor_tensor(e2[:], qview[:, :, 0], mview[:, :, 1], ADD)
                nc.vector.tensor_scalar(e2[:], e2[:], 1.0 / N_CORES, None, MUL)
                m1sq = cst.tile([128, HC], F32)
                nc.vector.tensor_tensor(m1sq[:], m1[:], m1[:], MUL)
                v1 = cst.tile([128, HC], F32)
                nc.vector.tensor_tensor(v1[:], e2[:], m1sq[:], SUB)
                sd1 = cst.tile([128, HC], F32)
                nc.vector.tensor_scalar(sd1[:], v1[:], 1.0, EPS, MUL, ADD)
                nc.scalar.sqrt(sd1[:], sd1[:])
                corr = cst.tile([128, HC], F32)
                nc.vector.tensor_tensor(corr[:], corr_pre[:], sd1[:], MUL)
                posT = cst.tile([128, HC], F32)   # threshold for is_gt
                negT = cst.tile([128, HC], F32)   # -threshold for ACT Sign bias
                nc.vector.tensor_tensor(posT[:], m1[:], corr[:], SUB)
                nc.vector.tensor_scalar(negT[:], posT[:], -1.0, None, MUL)

            # ---------------- phase B ----------------
            # s8 codes per slab, byte-paired for fp8 DoubleRow mm2; all
            # chunks {+-.5}: h0/h1/h2 on DVE, h3 on Pool (w2 = +-2).
            # mm2: 2 DR matmuls per bc.  Evac PSUM->h2T on ACT with
            # accum_out giving SUM(o) for free; SUMSQ(o) via ACT
            # Square+accum (12 bc) and DVE scalar_tensor_tensor (4 bc).
            SQF = mybir.ActivationFunctionType.Square
            with nc.named_scope("phaseB"):
                SLAB = 2048
                NSL = B_LOC // SLAB            # 4 slabs of 4 bc
                for sl in range(NSL):
                    ss = sl * SLAB
                    prt = []
                    for pi, (lo, hi) in enumerate(((0, 1), (2, 3))):
                        pt = s8p.tile([128, SLAB], U16, tag=f"s8{pi}")
                        v = pt[:].bitcast(FP8).rearrange("p (n j) -> p n j", j=2)
                        for j, h in ((0, lo), (1, hi)):
                            eng = nc.gpsimd if h == 3 else nc.vector
                            eng.tensor_scalar(
                                v[:, :, j], h1s[:, h, ss:ss + SLAB],
                                posT[:, h:h + 1], 0.5, GT, SUB)
                        prt.append(pt)
                    for j4 in range(4):
                        bc = sl * 4 + j4
                        bs = bc * BC
                        mp2 = psm.tile([OUTP, BC], F32, tag="mm")
                        for pi in range(2):
                            rhs = prt[pi][:, j4 * BC:(j4 + 1) * BC] \
                                .bitcast(FP8).rearrange("p (n j) -> p j n", j=2)
                            nc.tensor.matmul(
                                mp2[:], w2sT[:, pi, :, :], rhs,
                                start=(pi == 0), stop=(pi == 1),
                                perf_mode=DR)
                        nc.scalar.activation(h2T[:, bs:bs + BC], mp2[:OUT, :], IDENT,
                                             accum_out=s2sum[:, bc:bc + 1])
                        if j4 == 3:
                            hv = h2T[:, bs:bs + BC]
                            nc.vector.scalar_tensor_tensor(
                                scrD[:], hv, 1.0, hv, MUL, MUL,
                                accum_out=s2sq[:, bc:bc + 1])
                        else:
                            nc.scalar.activation(scrA[:], mp2[:OUT, :], SQF,
                                                 accum_out=s2sq[:, bc:bc + 1])

            agg2 = cst.tile([OUT, 2], F32)
            loc2 = dpool.tile([OUT, 2], F32)
            gat2 = dpool.tile([OUT * N_CORES, 2], F32)
            ga2 = cst.tile([OUT, N_CORES, 2], F32)
            with tc.high_priority():
                nc.vector.tensor_reduce(agg2[:, 0:1], s2sum[:],
                                        mybir.AxisListType.X, ADD)
                nc.vector.tensor_reduce(agg2[:, 1:2], s2sq[:],
                                        mybir.AxisListType.X, ADD)
                nc.gpsimd.dma_start(out=loc2[:], in_=agg2[:])
                nc.gpsimd.collective_compute(
                    "AllGather", mybir.AluOpType.bypass,
                    ins=[loc2.opt()], outs=[gat2.opt()],
                    replica_groups=[list(range(N_CORES))])
                nc.sync.dma_start(out=ga2[:], in_=gat2[:].rearrange("(c p) s -> p c s", p=OUT))

            with nc.named_scope("combine2"):
                # global SUM/SUMSQ -> mean/var -> affine params
                tot2 = cst.tile([OUT, 2], F32)
                nc.vector.tensor_reduce(tot2[:], ga2[:].rearrange("p c s -> p s c"),
                                        mybir.AxisListType.X, ADD)
                m2 = cst.tile([OUT, 1], F32)
                nc.vector.tensor_scalar(m2[:], tot2[:, 0:1], 1.0 / B, None, MUL)
                e22 = cst.tile([OUT, 1], F32)
                nc.vector.tensor_scalar(e22[:], tot2[:, 1:2], 1.0 / B, None, MUL)
                m2sq = cst.tile([OUT, 1], F32)
                nc.vector.tensor_tensor(m2sq[:], m2[:], m2[:], MUL)
                v2 = cst.tile([OUT, 1], F32)
                nc.vector.tensor_tensor(v2[:], e22[:], m2sq[:], SUB)
                sd2 = cst.tile([OUT, 1], F32)
                nc.vector.tensor_scalar(sd2[:], v2[:], 1.0, EPS, MUL, ADD)
                nc.scalar.sqrt(sd2[:], sd2[:])
                r2 = cst.tile([OUT, 1], F32)
                nc.vector.reciprocal(r2[:], sd2[:])
                scale2 = cst.tile([OUT, 1], F32)
                nc.vector.tensor_tensor(scale2[:], r2[:], g2c[:], MUL)
                shift2 = cst.tile([OUT, 1], F32)
                nc.vector.tensor_tensor(shift2[:], m2[:], scale2[:], MUL)
                nc.vector.tensor_tensor(shift2[:], b2c[:], shift2[:], SUB)

            # final affine split across DVE/ACT/Pool (rate-proportional),
            # then output DMA split across 4 queues.
            A0, A1, A2 = 4096, 6656, 8192
            nc.vector.tensor_scalar(h2T[:, 0:A0], h2T[:, 0:A0],
                                    scale2[:], shift2[:], MUL, ADD)
            nc.scalar.activation(h2T[:, A0:A1], h2T[:, A0:A1],
                                 IDENT, bias=shift2[:], scale=scale2[:])
            nc.gpsimd.tensor_scalar(h2T[:, A1:A2], h2T[:, A1:A2],
                                    scale2[:], shift2[:], MUL, ADD)
            OQ = [nc.sync, nc.gpsimd, nc.scalar]
            OSL = [(0, 2731), (2731, 5462), (5462, 8192)]
            for q, (ss, se) in enumerate(OSL):
                OQ[q].dma_start(out=o_out.ap()[:, ss:se], in_=h2T[:, ss:se])

    nc.compile()
    _cache["nc"] = nc
    return nc


def kernel(x, W1, W2, g1, b1, g2, b2, _trace=False):
    nc = build()
    x = np.ascontiguousarray(np.asarray(x, dtype=np.float32))
    in_maps = []
    for c in range(N_CORES):
        in_maps.append({
            "x": x[c * B_LOC:(c + 1) * B_LOC],
            "w1": np.asarray(W1, np.float32),
            "w2": np.asarray(W2, np.float32),
            "g1": np.asarray(g1, np.float32),
            "b1": np.asarray(b1, np.float32),
            "g2": np.asarray(g2, np.float32),
            "b2": np.asarray(b2, np.float32),
        })
    res = bass_utils.run_bass_kernel_spmd(nc, in_maps, core_ids=list(range(N_CORES)),
                                          trace=_trace)
    out = np.concatenate([np.ascontiguousarray(r["o_out"].T) for r in res.results], axis=0)
    if _trace:
        kernel.last_results = res
    return out


# revision 24
# speedup vs baseline: 1.3633x; 1.3633x over previous
"""Trainium2 Bass kernel for the binarized 2-layer MLP (nn_FC_small).

Network (reference semantics):
    h  = sign(x) @ sign(W1).T            # [B, 512], B = 65536, in = 768
    h  = batchnorm(h, g1, b1)            # training-mode, full-batch stats
    h  = clip(h, -1, 1)                  # hardtanh (sign-preserving)
    o  = sign(h) @ sign(W2).T            # [B, 10]
    o  = batchnorm(o, g2, b2)

Key identities:
  * sign(clip(z)) == sign(z); with g>0: sign(BN(h)) == sign(h - T),
    T = mu - b*sd/g.
  * Codes: {-.5,+.5} via DVE/Pool dual-op tensor_scalar ((x>0) - 0.5,
    weights +-2) or +-1 via ACT sign (weights +-1). Both give exact
    sign(x)*sign(w) products -- no per-column constants at all.

Performance structure (per core, batch 8192; engine budgets vs the
~76us x-stream DMA floor):
  - x streamed RAW f32 on sync+gpsimd queues; stage pool 4 bufs so the
    stream never stalls on a consumer hiccup.
  - fp8 codes in natural layout with byte-strided outs: byte pairs
    (feat f, feat f+384) form uint16 words; PE transposes on the uint16
    view feed fp8 DoubleRow mm1 directly.  DVE codes byte0 (dual-op,
    2x SBUF mode ~13us), ACT codes byte1 (sign, ~21us).
  - tp-evac PSUM->SBUF all on DVE (uint16 2x mode, ~13us).
  - mm1 evac PSUM->h1s f16 all on ACT (~32us).
  - bn_stats on the f16 h1s copy, [128,1024] per h-block (~36us DVE),
    keeping DVE ~62us < DMA floor.
  - Stats (mean,var) AllGather with a tight, high-priority launch chain;
    ncfw warmed up by a dummy AllGather at kernel start.
  - Phase B: s8 codes split DVE/ACT/Pool into fp8 byte pairs; mm2 is
    fp8 DoubleRow (2 matmuls per bc); mm2 evac via DMA; bn_stats per
    2048-slab; final affine split across DVE/ACT/Pool and output DMA
    split over 4 queues.

Sharding: data-parallel over batch across 8 cores (8192 rows each).
Output returned transposed [10, 8192] per core; host concatenates + .T.
"""
import numpy as np

import concourse.bass as bass
import concourse.bacc as bacc
import concourse.tile as tile
import concourse.mybir as mybir
from concourse import bass_utils
from concourse.masks import make_identity

F32 = mybir.dt.float32
F16 = mybir.dt.float16
BF16 = mybir.dt.bfloat16
FP8 = mybir.dt.float8e4
U16 = mybir.dt.uint16
GT = mybir.AluOpType.is_gt
MUL = mybir.AluOpType.mult
ADD = mybir.AluOpType.add
SUB = mybir.AluOpType.subtract
DR = mybir.MatmulPerfMode.DoubleRow
IDENT = mybir.ActivationFunctionType.Identity
SIGN = mybir.ActivationFunctionType.Sign

N_CORES = 8
B = 65536
IND, HID, OUT = 768, 512, 10
B_LOC = B // N_CORES          # 8192
BC = 512                      # batch chunk
NBC = B_LOC // BC             # 16
NT = 3                        # pair-plane tiles (384 pairs = 3*128)
HC = HID // 128               # 4 hid-chunks
EPS = 1e-5

_cache = {}


def build():
    if "nc" in _cache:
        return _cache["nc"]
    nc = bacc.Bacc("TRN2", target_bir_lowering=False, debug=False,
                   num_devices=N_CORES)
    x = nc.dram_tensor("x", [B_LOC, IND], F32, kind="ExternalInput")
    w1 = nc.dram_tensor("w1", [HID, IND], F32, kind="ExternalInput")
    w2 = nc.dram_tensor("w2", [OUT, HID], F32, kind="ExternalInput")
    g1 = nc.dram_tensor("g1", [HID], F32, kind="ExternalInput")
    b1 = nc.dram_tensor("b1", [HID], F32, kind="ExternalInput")
    g2 = nc.dram_tensor("g2", [OUT], F32, kind="ExternalInput")
    b2 = nc.dram_tensor("b2", [OUT], F32, kind="ExternalInput")
    o_out = nc.dram_tensor("o_out", [OUT, B_LOC], F32, kind="ExternalOutput")

    with tile.TileContext(nc) as tc:
        with (
            tc.tile_pool(name="cst", bufs=1) as cst,
            tc.tile_pool(name="stage", bufs=3) as stage,
            tc.tile_pool(name="wpool", bufs=1) as wpool,
            tc.tile_pool(name="code", bufs=2) as codep,
            tc.tile_pool(name="s8p", bufs=2) as s8p,
            tc.tile_pool(name="pst", bufs=2, space="PSUM") as pst,    # transposes
            tc.tile_pool(name="psm", bufs=3, space="PSUM") as psm,    # mm1/mm2
            tc.tile_pool(name="dram", bufs=1, space="DRAM") as dpool,
        ):
            ident = cst.tile([128, 128], BF16)
            make_identity(nc, ident[:])
            bias_m1 = cst.tile([128, 1], F32)
            nc.vector.memset(bias_m1[:], -1.0)
            bias_m2 = cst.tile([128, 1], F32)
            nc.vector.memset(bias_m2[:], -2.0)
            wup_sb = cst.tile([128, 1], F32)

            # x-chunk DMAs alternate sync / gpsimd issue queues; both are
            # otherwise light, and a single queue bubbles the stream on
            # descriptor-generation gaps.
            def x_dma(bc_, xt_):
                eng = nc.gpsimd if bc_ % 2 == 0 else nc.sync
                eng.dma_start(
                    out=xt_[:],
                    in_=x.ap()[bc_ * BC:(bc_ + 1) * BC].rearrange("(s p) f -> p s f", p=128))

            xcs = {}
            for pc in range(3):
                xt = stage.tile([128, 4, IND], F32, tag="xc")
                x_dma(pc, xt)
                xcs[pc] = xt

            # codes: byte0 (feat 0..383)   = {0,1} is_gt on DVE (w' = +-2,
            #        constant absorbed by the BN1 batch mean)
            #        byte1 (feat 384..767) = sign(x) on ACT (w' = +-1)
            def make_codes(cdt_, xc_):
                cdtv = cdt_[:].rearrange("p s (q j) -> p s q j", j=2)
                nc.vector.tensor_scalar(
                    cdtv[:, :, :, 0], xc_[:, :, 0:384], 0.0, None, GT)
                nc.scalar.sign(cdtv[:, :, :, 1], xc_[:, :, 384:768])

            cds = {}
            for pc in range(2):
                cdt = codep.tile([128, 4, IND], FP8, tag="cd")
                make_codes(cdt, xcs[pc])
                cds[pc] = cdt

            # ---------------- weight prep ----------------
            # W1 [512, 768] -> {0,1} codes -> pair-layout fp8
            # w1sT[ki, t, ko, hid]: ko=0 -> +-2 (scale 4 bias -2);
            #                       ko=1 -> +-1 (scale 2 bias -1)
            w1c = wpool.tile([128, 4, IND], F32)
            nc.sync.dma_start(out=w1c[:], in_=w1.ap().rearrange("(c p) f -> p c f", p=128))
            w1b = wpool.tile([128, 4, IND], BF16)
            nc.vector.tensor_scalar(w1b[:], w1c[:], 0.0, None, GT)
            w1sT = cst.tile([128, NT, 2, HID], FP8)   # 3 KB/part
            for k in range(6):
                t, ko = (k, 0) if k < 3 else (k - 3, 1)
                sc, bi = (4.0, bias_m2) if k < 3 else (2.0, bias_m1)
                pw = psm.tile([128, HC, 128], BF16, tag="mm")
                for c in range(HC):
                    nc.tensor.transpose(pw[:, c, :], w1b[:, c, k * 128:(k + 1) * 128], ident[:])
                nc.scalar.activation(w1sT[:, t, ko, :], pw[:].rearrange("p c f -> p (c f)"),
                                     IDENT, bias=bi[:], scale=sc)

            # ---------------- persistent buffers ----------------
            xT8 = cst.tile([128, NT, B_LOC], U16, tag="bigx")   # 48 KB/part
            h1s = cst.tile([128, HC, B_LOC], F16, tag="bigh")   # 64 KB/part
            st1 = cst.tile([128, HC, NBC * 6], F32)
            h2T = cst.tile([OUT, B_LOC], F32, tag="bigx")       # reuses xT8 slab
            s2sum = cst.tile([OUT, NBC], F32)
            s2sq = cst.tile([OUT, NBC], F32)
            scrA = cst.tile([OUT, BC], F32)
            scrD = cst.tile([OUT, BC], F32)

            # warm-up collective: pays the ncfw cold-start barrier during
            # phase A.  The prefetch DMAs above are already enqueued on the
            # gpsimd queue, so the doorbell stall overlaps chunk 0-2 compute.
            wloc = dpool.tile([128, 1], F32)
            wgat = dpool.tile([128 * N_CORES, 1], F32)
            with tc.high_priority():
                nc.vector.memset(wup_sb[:], 0.0)
                nc.sync.dma_start(out=wloc[:], in_=wup_sb[:])
                nc.gpsimd.collective_compute(
                    "AllGather", mybir.AluOpType.bypass,
                    ins=[wloc.opt()], outs=[wgat.opt()],
                    replica_groups=[list(range(N_CORES))])

            # ---------------- phase A ----------------
            # per chunk: DMA f32 -> codes (byte-strided fp8) -> uint16 pair
            # transposes -> DVE evac into xT8.  mm1 h-blocks of the PREVIOUS
            # group are emitted between transpose halves so real-matmul
            # activity never pauses long enough for the PE HAM clock gate
            # to re-throttle.  Stats run on the f16 h1s copy (not PSUM).
            def mm_h_block(g, h):
                bs0 = 2 * g * BC
                mp = psm.tile([128, 2, BC], F32, tag="mm")
                for t in range(NT):
                    lw = w1sT[:, t, :, h * 128:(h + 1) * 128]
                    for j in range(2):
                        rhs = xT8[:, t, bs0 + j * BC: bs0 + (j + 1) * BC] \
                            .bitcast(FP8).rearrange("p (n j) -> p j n", j=2)
                        nc.tensor.matmul(
                            mp[:, j, :], lw, rhs,
                            start=(t == 0), stop=(t == NT - 1),
                            perf_mode=DR)
                mpw = mp[:].rearrange("p j n -> p (j n)")
                nc.scalar.copy(h1s[:, h, bs0:bs0 + 2 * BC], mpw)
                for j in range(2):
                    bc2 = 2 * g + j
                    nc.vector.bn_stats(st1[:, h, bc2 * 6:(bc2 + 1) * 6],
                                       h1s[:, h, bs0 + j * BC:bs0 + (j + 1) * BC])

            with nc.named_scope("phaseA"):
                for grp in range(NBC // 2):
                    hb = 0
                    for half2 in range(2):
                        bc = 2 * grp + half2
                        bs = bc * BC
                        if bc in xcs:
                            xc = xcs.pop(bc)
                        else:
                            xc = stage.tile([128, 4, IND], F32, tag="xc")
                            x_dma(bc, xc)
                        if bc + 2 < NBC and bc + 2 not in xcs:
                            # keep the DMA queue primed ~3 chunks ahead
                            nxt = stage.tile([128, 4, IND], F32, tag="xc")
                            x_dma(bc + 2, nxt)
                            xcs[bc + 2] = nxt
                        if bc in cds:
                            cd = cds.pop(bc)
                        else:
                            cd = codep.tile([128, 4, IND], FP8, tag="cd")
                            make_codes(cd, xc)
                        for half in range(2):
                            # mm block FIRST: it is ready to run while the
                            # transposes below wait on this chunk's DMA and
                            # codes — avoids head-of-line blocking on the
                            # PE's strict-FIFO queue.
                            if grp > 0:
                                mm_h_block(grp - 1, hb)
                                hb += 1
                            tp = pst.tile([128, NT, 2, 128], BF16, tag="tp")
                            for t in range(NT):
                                for s2 in range(2):
                                    s = 2 * half + s2
                                    cdu = cd[:, s, :].bitcast(BF16)
                                    nc.tensor.transpose(
                                        tp[:, t, s2, :],
                                        cdu[:, t * 128:(t + 1) * 128],
                                        ident[:])
                            # one evac per half-chunk, uint16 2x mode on DVE
                            ev_out = xT8[:, :, bs + half * 256: bs + half * 256 + 256] \
                                .rearrange("p t (s b) -> p t s b", s=2).bitcast(BF16)
                            nc.vector.tensor_copy(ev_out, tp[:])
            # ---------------- local stats -> AllGather ----------------
            agg1 = cst.tile([128, HC, 2], F32)
            loc1 = dpool.tile([128, HC * 2], F32)
            gat1 = dpool.tile([128 * N_CORES, HC * 2], F32)
            ga1 = cst.tile([128, N_CORES, HC * 2], F32)
            with nc.named_scope("phaseA"):
                for h in range(HC):
                    mm_h_block(NBC // 2 - 1, h)
                    with tc.high_priority():
                        nc.vector.bn_aggr(agg1[:, h, :],
                                          st1[:, h, :].rearrange("p (n s) -> p n s", s=6))
            with tc.high_priority():
                nc.gpsimd.dma_start(out=loc1[:], in_=agg1[:].rearrange("p c s -> p (c s)"))
                nc.gpsimd.collective_compute(
                    "AllGather", mybir.AluOpType.bypass,
                    ins=[loc1.opt()], outs=[gat1.opt()],
                    replica_groups=[list(range(N_CORES))])
                nc.sync.dma_start(out=ga1[:], in_=gat1[:].rearrange("(c p) s -> p c s", p=128))

            # W2 prep + g/b vectors (overlaps phase A / collective)
            # chunk -> (pair, ko): c0->(0,0) +-2; c1->(0,1) +-1;
            #                      c2->(1,0) +-1; c3->(1,1) +-2
            w2n = cst.tile([OUT, HID], F32)
            nc.sync.dma_start(out=w2n[:], in_=w2.ap())
            w2b = cst.tile([OUT, HID], BF16)
            nc.vector.tensor_scalar(w2b[:], w2n[:], 0.0, None, GT)
            # all four chunks coded {+-.5} (h0/h1/h2 DVE, h3 Pool) -> w2 +-2.
            # Stationary padded to 16 cols (DoubleRow needs >=16); pad = 0.
            OUTP = 16
            w2sT = cst.tile([128, 2, 2, OUTP], FP8)
            nc.vector.memset(w2sT[:], 0.0)
            W2MAP = [(0, 0, 4.0), (0, 1, 4.0), (1, 0, 4.0), (1, 1, 4.0)]
            for c in range(HC):
                pw2 = psm.tile([128, OUT], BF16, tag="mm")
                nc.tensor.transpose(pw2[:], w2b[:, c * 128:(c + 1) * 128], ident[:OUT, :OUT])
                pr, ko, sc = W2MAP[c]
                bi = bias_m2 if sc == 4.0 else bias_m1
                nc.scalar.activation(w2sT[:, pr, ko, 0:OUT], pw2[:], IDENT, bias=bi[:], scale=sc)
            g1c = cst.tile([128, HC], F32)
            b1c = cst.tile([128, HC], F32)
            for c in range(HC):
                nc.sync.dma_start(out=g1c[:, c:c + 1], in_=g1.ap()[c * 128:(c + 1) * 128])
                nc.sync.dma_start(out=b1c[:, c:c + 1], in_=b1.ap()[c * 128:(c + 1) * 128])
            g2c = cst.tile([OUT, 1], F32)
            b2c = cst.tile([OUT, 1], F32)
            nc.sync.dma_start(out=g2c[:], in_=g2.ap())
            nc.sync.dma_start(out=b2c[:], in_=b2.ap())
            # corr_pre = b1/g1 does not depend on the collective result
            ig1 = cst.tile([128, HC], F32)
            nc.vector.reciprocal(ig1[:], g1c[:])
            corr_pre = cst.tile([128, HC], F32)
            nc.vector.tensor_tensor(corr_pre[:], b1c[:], ig1[:], MUL)

            # combine: mean_tot = avg(mean_c); var_tot = avg(var_c + mean_c^2) - mean_tot^2
            with nc.named_scope("combine1"):
                q1 = cst.tile([128, N_CORES, HC * 2], F32)
                nc.vector.tensor_tensor(q1[:], ga1[:], ga1[:], MUL)
                msum = cst.tile([128, HC * 2], F32)
                qsum = cst.tile([128, HC * 2], F32)
                nc.vector.tensor_reduce(msum[:], ga1[:].rearrange("p c s -> p s c"),
                                        mybir.AxisListType.X, ADD)
                nc.vector.tensor_reduce(qsum[:], q1[:].rearrange("p c s -> p s c"),
                                        mybir.AxisListType.X, ADD)
                m1 = cst.tile([128, HC], F32)
                mview = msum[:].rearrange("p (c s) -> p c s", s=2)
                qview = qsum[:].rearrange("p (c s) -> p c s", s=2)
                nc.vector.tensor_scalar(m1[:], mview[:, :, 0], 1.0 / N_CORES, None, MUL)
                e2 = cst.tile([128, HC], F32)
                nc.vector.tensor_tensor(e2[:], qview[:, :, 0], mview[:, :, 1], ADD)
                nc.vector.tensor_scalar(e2[:], e2[:], 1.0 / N_CORES, None, MUL)
                m1sq = cst.tile([128, HC], F32)
                nc.vector.tensor_tensor(m1sq[:], m1[:], m1[:], MUL)
                v1 = cst.tile([128, HC], F32)
                nc.vector.tensor_tensor(v1[:], e2[:], m1sq[:], SUB)
                sd1 = cst.tile([128, HC], F32)
                nc.vector.tensor_scalar(sd1[:], v1[:], 1.0, EPS, MUL, ADD)
                nc.scalar.sqrt(sd1[:], sd1[:])
                corr = cst.tile([128, HC], F32)
                nc.vector.tensor_tensor(corr[:], corr_pre[:], sd1[:], MUL)
                posT = cst.tile([128, HC], F32)   # threshold for is_gt
                negT = cst.tile([128, HC], F32)   # -threshold for ACT Sign bias
                nc.vector.tensor_tensor(posT[:], m1[:], corr[:], SUB)
                nc.vector.tensor_scalar(negT[:], posT[:], -1.0, None, MUL)

            # ---------------- phase B ----------------
            # s8 codes per slab, byte-paired for fp8 DoubleRow mm2; all
            # chunks {+-.5}: h0/h1/h2 on DVE, h3 on Pool (w2 = +-2).
            # mm2: 2 DR matmuls per bc.  Evac PSUM->h2T on ACT with
            # accum_out giving SUM(o) for free; SUMSQ(o) via ACT
            # Square+accum (12 bc) and DVE scalar_tensor_tensor (4 bc).
            SQF = mybir.ActivationFunctionType.Square
            with nc.named_scope("phaseB"):
                SLAB = 2048
                NSL = B_LOC // SLAB            # 4 slabs of 4 bc
                for sl in range(NSL):
                    ss = sl * SLAB
                    prt = []
                    for pi, (lo, hi) in enumerate(((0, 1), (2, 3))):
                        pt = s8p.tile([128, SLAB], U16, tag=f"s8{pi}")
                        v = pt[:].bitcast(FP8).rearrange("p (n j) -> p n j", j=2)
                        for j, h in ((0, lo), (1, hi)):
                            # {0,1} codes; the +-2 weight constant cancels in
                            # the BN2 batch mean.  (Dual-op with a scalar
                            # pointer is an 18 cyc/elem slow path on HW.)
                            eng = nc.gpsimd if h == 3 else nc.vector
                            eng.tensor_scalar(
                                v[:, :, j], h1s[:, h, ss:ss + SLAB],
                                posT[:, h:h + 1], None, GT)
                        prt.append(pt)
                    for j4 in range(4):
                        bc = sl * 4 + j4
                        bs = bc * BC
                        mp2 = psm.tile([OUTP, BC], F32, tag="mm")
                        for pi in range(2):
                            rhs = prt[pi][:, j4 * BC:(j4 + 1) * BC] \
                                .bitcast(FP8).rearrange("p (n j) -> p j n", j=2)
                            nc.tensor.matmul(
                                mp2[:], w2sT[:, pi, :, :], rhs,
                                start=(pi == 0), stop=(pi == 1),
                                perf_mode=DR)
                        nc.scalar.activation(h2T[:, bs:bs + BC], mp2[:OUT, :], IDENT,
                                             accum_out=s2sum[:, bc:bc + 1])
                        if j4 == 3:
                            hv = h2T[:, bs:bs + BC]
                            nc.vector.scalar_tensor_tensor(
                                scrD[:], hv, 1.0, hv, MUL, MUL,
                                accum_out=s2sq[:, bc:bc + 1])
                        else:
                            nc.scalar.activation(scrA[:], mp2[:OUT, :], SQF,
                                                 accum_out=s2sq[:, bc:bc + 1])

            agg2 = cst.tile([OUT, 2], F32)
            loc2 = dpool.tile([OUT, 2], F32)
            gat2 = dpool.tile([OUT * N_CORES, 2], F32)
            ga2 = cst.tile([OUT, N_CORES, 2], F32)
            with tc.high_priority():
                nc.vector.tensor_reduce(agg2[:, 0:1], s2sum[:],
                                        mybir.AxisListType.X, ADD)
                nc.vector.tensor_reduce(agg2[:, 1:2], s2sq[:],
                                        mybir.AxisListType.X, ADD)
                nc.gpsimd.dma_start(out=loc2[:], in_=agg2[:])
                nc.gpsimd.collective_compute(
                    "AllGather", mybir.AluOpType.bypass,
                    ins=[loc2.opt()], outs=[gat2.opt()],
                    replica_groups=[list(range(N_CORES))])
                nc.sync.dma_start(out=ga2[:], in_=gat2[:].rearrange("(c p) s -> p c s", p=OUT))

            with nc.named_scope("combine2"):
                # global SUM/SUMSQ -> mean/var -> affine params
                tot2 = cst.tile([OUT, 2], F32)
                nc.vector.tensor_reduce(tot2[:], ga2[:].rearrange("p c s -> p s c"),
                                        mybir.AxisListType.X, ADD)
                m2 = cst.tile([OUT, 1], F32)
                nc.vector.tensor_scalar(m2[:], tot2[:, 0:1], 1.0 / B, None, MUL)
                e22 = cst.tile([OUT, 1], F32)
                nc.vector.tensor_scalar(e22[:], tot2[:, 1:2], 1.0 / B, None, MUL)
                m2sq = cst.tile([OUT, 1], F32)
                nc.vector.tensor_tensor(m2sq[:], m2[:], m2[:], MUL)
                v2 = cst.tile([OUT, 1], F32)
                nc.vector.tensor_tensor(v2[:], e22[:], m2sq[:], SUB)
                sd2 = cst.tile([OUT, 1], F32)
                nc.vector.tensor_scalar(sd2[:], v2[:], 1.0, EPS, MUL, ADD)
                nc.scalar.sqrt(sd2[:], sd2[:])
                r2 = cst.tile([OUT, 1], F32)
                nc.vector.reciprocal(r2[:], sd2[:])
                scale2 = cst.tile([OUT, 1], F32)
                nc.vector.tensor_tensor(scale2[:], r2[:], g2c[:], MUL)
                shift2 = cst.tile([OUT, 1], F32)
                nc.vector.tensor_tensor(shift2[:], m2[:], scale2[:], MUL)
                nc.vector.tensor_tensor(shift2[:], b2c[:], shift2[:], SUB)

            # final affine split across DVE/ACT/Pool (rate-proportional),
            # then output DMA split across 4 queues.
            A0, A1, A2 = 4096, 6656, 8192
            nc.vector.tensor_scalar(h2T[:, 0:A0], h2T[:, 0:A0],
                                    scale2[:], shift2[:], MUL, ADD)
            nc.scalar.activation(h2T[:, A0:A1], h2T[:, A0:A1],
                                 IDENT, bias=shift2[:], scale=scale2[:])
            nc.gpsimd.tensor_scalar(h2T[:, A1:A2], h2T[:, A1:A2],
                                    scale2[:], shift2[:], MUL, ADD)
            OQ = [nc.sync, nc.gpsimd, nc.scalar]
            OSL = [(0, 2731), (2731, 5462), (5462, 8192)]
            for q, (ss, se) in enumerate(OSL):
                OQ[q].dma_start(out=o_out.ap()[:, ss:se], in_=h2T[:, ss:se])

    nc.compile()
    _cache["nc"] = nc
    return nc


def kernel(x, W1, W2, g1, b1, g2, b2, _trace=False):
    nc = build()
    x = np.ascontiguousarray(np.asarray(x, dtype=np.float32))
    in_maps = []
    for c in range(N_CORES):
        in_maps.append({
            "x": x[c * B_LOC:(c + 1) * B_LOC],
            "w1": np.asarray(W1, np.float32),
            "w2": np.asarray(W2, np.float32),
            "g1": np.asarray(g1, np.float32),
            "b1": np.asarray(b1, np.float32),
            "g2": np.asarray(g2, np.float32),
            "b2": np.asarray(b2, np.float32),
        })
    res = bass_utils.run_bass_kernel_spmd(nc, in_maps, core_ids=list(range(N_CORES)),
                                          trace=_trace)
    out = np.concatenate([np.ascontiguousarray(r["o_out"].T) for r in res.results], axis=0)
    if _trace:
        kernel.last_results = res
    return out
